# revision 9
# baseline (speedup 1.0000x reference)
"""Dilated-attention Trainium2 kernel (8 NeuronCores, SPMD), bf16/fp8 edition.

Problem: x [4, 16384, 768] f32. Per 512-token segment, take every 2nd
position (dilation 2) -> 128 independent segments of [256, 768]; per-segment
self-attention out = softmax(xs @ xs.T / sqrt(768)) @ xs; output [4, 8192, 768].

Sharding: 128 (batch x segment) attention problems are fully independent ->
16 segments per core, no cross-core communication. The dilation gather, the
position-major -> partition-major permutation, the bf16/fp8 casts and the
final numerator/denominator divide are host-side (pure data movement /
elementwise; overall relative error ~2.3e-3, well under the 2e-2 gate).

Device inputs per core (all per-partition contiguous in DRAM):
  x   [128 p, 16 s, 2 t, 772] bf16 -- position-major, position = t*128+p,
      cols 768:772 hold literal 1.0 (fused softmax denominator)
  xt  [128 dp, 16 s, 3 j, 2 c, 256 pos] fp8e4m3 -- feature-major transposed
      copy interleaved for DoubleRow (feature = j*256 + c*128 + dp), Q/K
      side only; fp8 only perturbs attention logits (rel err stays 2.3e-3)
Output y [128 p, 16 s, 2 t, 769] bf16: cols 0:768 = un-normalized E @ [X|1]
numerator, col 768 = softmax denominator; host divides.

v2 schedule (full SBUF residency):
  All 16 segments' inputs, exps and outputs are resident in SBUF at once
  (~139 KB/partition), so no tile recycling ever gates a DMA. All input
  DMAs are issued as the first instructions of the two HWDGE rings
  (sync: segs 0,1,4,5,8,9,12,13; scalar: 2,3,6,7,10,11,14,15), finely
  split at the head so compute starts on segment 0 as early as possible.
  Output stores go per-segment on the gpsimd SWDGE ring (segs 0-11) and
  the tail on the then-idle HWDGE rings (sync: 12,14; scalar: 13,15).
  PSUM evictions are split scalar/vector/gpsimd to keep every engine
  below the DMA-roofline span.
"""

import numpy as np
import ml_dtypes

import concourse.bass as bass
import concourse.mybir as mybir
import concourse.tile as tile
from concourse.bass_utils import run_bass_kernel_spmd

F32 = mybir.dt.float32
BF16 = mybir.dt.bfloat16
FP8 = mybir.dt.float8e4

B, S_FULL, D = 4, 16384, 768
SEG, DIL = 512, 2
L = SEG // DIL                      # 256 positions per dilated segment
NSEG = B * (S_FULL // SEG)          # 128 segments total
NCORE = 8
SEG_PER_CORE = NSEG // NCORE        # 16
KT = L // 128                       # 2 position tiles per segment
DT = D // 128                       # 6 feature tiles
DW = D + 4                          # free pitch (cols 768:772 = 1.0)
SCALE = 1.0 / float(np.sqrt(D))
MAXB = 2                            # segments per compute batch / input group
NGRP = SEG_PER_CORE // MAXB         # 8 groups of 2 segments
OW = D + 1                          # output pitch: 768 numerator + denominator


def build_nc():
    nc = bass.Bass()
    x = nc.dram_tensor("x", [128, SEG_PER_CORE, KT, DW], BF16, kind="ExternalInput")
    xt = nc.dram_tensor(
        "xt", [128, SEG_PER_CORE, DT // 2, 2, L], FP8, kind="ExternalInput"
    )
    y = nc.dram_tensor("y", [128, SEG_PER_CORE, KT, OW], BF16, kind="ExternalOutput")
    Exp = mybir.ActivationFunctionType.Exp

    with tile.TileContext(nc) as tc:
        with (
            tc.tile_pool(name="xn", bufs=NGRP) as xn_pool,
            tc.tile_pool(name="xf", bufs=NGRP) as xf_pool,
            tc.tile_pool(name="e", bufs=SEG_PER_CORE) as e_pool,
            tc.tile_pool(name="osb", bufs=SEG_PER_CORE) as osb_pool,
            tc.tile_pool(name="ps", bufs=2, space="PSUM") as ps_pool,
        ):
            # static per-group input tiles (never recycled)
            xn_t = []
            xf_t = []
            for g in range(NGRP):
                xn_t.append(xn_pool.tile([128, MAXB, KT, DW], BF16, tag="xn", name=f"xn{g}"))
                xf_t.append(xf_pool.tile([128, MAXB, DT // 2, 2, L], FP8, tag="xf", name=f"xf{g}"))

            def issue_group(g):
                """Issue group g's input DMAs: even groups on the sync HWDGE
                ring, odd on scalar (xf as one 2-seg transfer, xn per-seg)."""
                eng = nc.sync if g % 2 == 0 else nc.scalar
                s0 = MAXB * g
                eng.dma_start(out=xf_t[g], in_=xt[:, s0 : s0 + MAXB])
                for k in range(MAXB):
                    eng.dma_start(out=xn_t[g][:, k], in_=x[:, s0 + k])

            # pre-loop: segments 0-3 alternate rings PER SEGMENT so both the
            # xf and xn of the earliest segments land as soon as possible
            # (one ring carrying xf0,xn0,xf1,xn1 serially would delay V0 by
            # ~3us). 8 transfers = exactly the HWDGE outstanding-sem window;
            # issuing more here would make later issue instructions block
            # their engine on the sem-window wait, which on scalar would
            # stall the exps queued behind them.
            for s in range(4):
                eng = nc.sync if s % 2 == 0 else nc.scalar
                eng.dma_start(out=xf_t[s // 2][:, s % 2], in_=xt[:, s])
                eng.dma_start(out=xn_t[s // 2][:, s % 2], in_=x[:, s])
            # remaining groups: paced 2 per batch, emitted AFTER the batch's
            # exps so scalar reaches them with the window-wait long satisfied
            issue_at = {0: (2, 3), 1: (4, 5), 2: (6, 7)}

            for g in range(NGRP):
                s0 = MAXB * g
                # ---- Q/K phase for both segments of the group
                es = []
                for sl in range(MAXB):
                    xfs = xf_t[g][:, sl]
                    sp = ps_pool.tile([128, 512], F32, tag="sp")
                    DR = mybir.MatmulPerfMode.DoubleRow
                    for kt in range(KT):
                        for j in range(DT // 2):
                            nc.tensor.matmul(
                                sp[:, kt * 256 : kt * 256 + 256],
                                xfs[:, j, :, kt * 128 : kt * 128 + 128],
                                xfs[:, j],
                                start=(j == 0),
                                stop=(j == DT // 2 - 1),
                                perf_mode=DR,
                                skip_group_check=(kt == 1),
                            )
                    e = e_pool.tile([128, 512], BF16, tag="e")
                    nc.scalar.activation(e[:], sp[:], Exp, scale=SCALE)
                    es.append(e)

                # paced input DMA issues (after exps, before V-phase copies)
                for gi in issue_at.get(g, ()):
                    issue_group(gi)

                # ---- V phase + eviction + per-segment store
                for sl in range(MAXB):
                    s = s0 + sl
                    e = es[sl]
                    xns = xn_t[g][:, sl]
                    osb = osb_pool.tile([128, KT, OW], BF16, tag="osb")
                    for qt in range(KT):
                        op0 = ps_pool.tile([128, 388], F32, tag="op0", bufs=3)
                        op1 = ps_pool.tile([128, 388], F32, tag="op1", bufs=3)
                        for kt in range(KT):
                            lhsT = e[:, kt * 256 + qt * 128 : kt * 256 + qt * 128 + 128]
                            nc.tensor.matmul(
                                op0[:, 0:384],
                                lhsT,
                                xns[:, kt, 0:384],
                                start=(kt == 0),
                                stop=(kt == KT - 1),
                            )
                            nc.tensor.matmul(
                                op1[:, 0:388],
                                lhsT,
                                xns[:, kt, 384:772],
                                start=(kt == 0),
                                stop=(kt == KT - 1),
                            )
                        dst = osb[:, qt]
                        if qt:
                            nc.scalar.copy(dst[:, 0:384], op0[:, 0:384])
                            nc.vector.tensor_copy(dst[:, 384:769], op1[:, 0:385])
                        else:
                            nc.vector.tensor_copy(dst[:, 0:384], op0[:, 0:384])
                            nc.vector.tensor_copy(dst[:, 384:769], op1[:, 0:385])

                    # per-segment store: SWDGE for the early segments (6 < the
                    # 8-deep SWDGE sem window, so no window stalls), the rest
                    # split across the HWDGE rings, whose FIFOs naturally
                    # finish their input backlog first
                    yv = y[:, s].rearrange("p t d -> p (t d)")
                    ov = osb.rearrange("p t d -> p (t d)")
                    if s < 6:
                        nc.gpsimd.dma_start(out=yv, in_=ov)
                    elif s % 2 == 0:
                        nc.sync.dma_start(out=yv, in_=ov)
                    else:
                        nc.scalar.dma_start(out=yv, in_=ov)
    return nc


def split_excess_waits(nc, max_waits=1):
    """This walrus build only encodes one sync wait per instruction; move
    excess waits onto preceding same-engine NOPs."""
    n_split = 0
    for fn in nc.m.functions:
        for blk in fn.blocks:
            insts = blk.instructions
            i = 0
            while i < len(insts):
                inst = insts[i]
                si = getattr(inst, "sync_info", None)
                waits = list(si.on_wait) if si and si.on_wait else []
                if len(waits) > max_waits:
                    nop = mybir.InstNoOp(name=f"I-waitsplit-{n_split}", ins=[], outs=[])
                    nop.engine = inst.engine
                    nop.sync_info = mybir.SyncInfo(
                        on_wait=waits[:max_waits], on_update=[]
                    )
                    inst.sync_info = mybir.SyncInfo(
                        on_wait=waits[max_waits:], on_update=list(si.on_update)
                    )
                    insts.insert(i, nop)
                    n_split += 1
                else:
                    i += 1
    return n_split


_NC = None


def _get_nc():
    global _NC
    if _NC is None:
        _NC = build_nc()
        split_excess_waits(_NC)
    return _NC


def shard_inputs(x):
    """Full x [4, 16384, 768] f32 -> 8 per-core dicts:
    x  [128, 16, 2, 772] bf16 (position-major + ones cols)
    xt [128, 16, 6, 2, 128] fp8e4m3 (feature-major)
    """
    xd = np.asarray(x).reshape(B, S_FULL // SEG, SEG, D)[:, :, ::DIL, :]
    xd = xd.reshape(NSEG, KT, 128, D)                 # [seg, t, p, d]
    xp = xd.transpose(2, 0, 1, 3)                     # [p, seg, t, d]
    xb = np.empty((128, NSEG, KT, DW), dtype=ml_dtypes.bfloat16)
    xb[..., 0:D] = xp.astype(ml_dtypes.bfloat16)
    xb[..., D:DW] = np.asarray(1.0, dtype=ml_dtypes.bfloat16)
    xt = (
        xb[..., 0:D]
        .reshape(128, NSEG, KT, DT // 2, 2, 128)      # [p, seg, t, j, c, dp]
        .transpose(5, 1, 3, 4, 2, 0)                  # [dp, seg, j, c, t, p]
        .reshape(128, NSEG, DT // 2, 2, L)            # [dp, seg, j, c, pos]
        .astype(ml_dtypes.float8_e4m3)
    )
    out = []
    for c in range(NCORE):
        sl = slice(SEG_PER_CORE * c, SEG_PER_CORE * (c + 1))
        out.append(
            {
                "x": np.ascontiguousarray(xb[:, sl]),
                "xt": np.ascontiguousarray(xt[:, sl]),
            }
        )
    return out


def assemble_output(results):
    ys = np.concatenate([results[c]["y"] for c in range(NCORE)], axis=1)
    ys = ys.astype(np.float32)                        # [p, seg, t, 769]
    num = ys[..., 0:D].transpose(1, 2, 0, 3)          # [seg, t, p, d]
    den = ys[..., D].transpose(1, 2, 0)[..., None]    # [seg, t, p, 1]
    out = num / den
    return np.ascontiguousarray(out.reshape(B, (S_FULL // SEG) * L, D))


def kernel(x):
    nc = _get_nc()
    in_maps = shard_inputs(x)
    core_ids = list(range(NCORE))
    # run twice: the first execution after a fresh NEFF load has been seen
    # returning unwritten output buffers; the repeat is cheap and reliable.
    run_bass_kernel_spmd(nc, in_maps, core_ids)
    res = run_bass_kernel_spmd(nc, in_maps, core_ids)
    return assemble_output(res.results)


# revision 11
# speedup vs baseline: 1.0037x; 1.0037x over previous
"""Dilated-attention Trainium2 kernel (8 NeuronCores, SPMD), bf16/fp8 edition.

Problem: x [4, 16384, 768] f32. Per 512-token segment, take every 2nd
position (dilation 2) -> 128 independent segments of [256, 768]; per-segment
self-attention out = softmax(xs @ xs.T / sqrt(768)) @ xs; output [4, 8192, 768].

Sharding: 128 (batch x segment) attention problems are fully independent ->
16 segments per core, no cross-core communication. The dilation gather, the
position-major -> partition-major permutation, the bf16/fp8 casts and the
final numerator/denominator divide are host-side (pure data movement /
elementwise; overall relative error ~2.3e-3, well under the 2e-2 gate).

Device inputs per core (all per-partition contiguous in DRAM):
  x   [128 p, 16 s, 2 t, 772] bf16 -- position-major, position = t*128+p,
      cols 768:772 hold literal 1.0 (fused softmax denominator)
  xt  [128 dp, 16 s, 3 j, 2 c, 256 pos] fp8e4m3 -- feature-major transposed
      copy interleaved for DoubleRow (feature = j*256 + c*128 + dp), Q/K
      side only; fp8 only perturbs attention logits (rel err stays 2.3e-3)
Output y [128 p, 16 s, 2 t, 769] bf16: cols 0:768 = un-normalized E @ [X|1]
numerator, col 768 = softmax denominator; host divides.

v2 schedule (full SBUF residency):
  All 16 segments' inputs, exps and outputs are resident in SBUF at once
  (~139 KB/partition), so no tile recycling ever gates a DMA. All input
  DMAs are issued as the first instructions of the two HWDGE rings
  (sync: segs 0,1,4,5,8,9,12,13; scalar: 2,3,6,7,10,11,14,15), finely
  split at the head so compute starts on segment 0 as early as possible.
  Output stores go per-segment on the gpsimd SWDGE ring (segs 0-11) and
  the tail on the then-idle HWDGE rings (sync: 12,14; scalar: 13,15).
  PSUM evictions are split scalar/vector/gpsimd to keep every engine
  below the DMA-roofline span.
"""

import numpy as np
import ml_dtypes

import concourse.bass as bass
import concourse.mybir as mybir
import concourse.tile as tile
from concourse.bass_utils import run_bass_kernel_spmd

F32 = mybir.dt.float32
BF16 = mybir.dt.bfloat16
FP8 = mybir.dt.float8e4

B, S_FULL, D = 4, 16384, 768
SEG, DIL = 512, 2
L = SEG // DIL                      # 256 positions per dilated segment
NSEG = B * (S_FULL // SEG)          # 128 segments total
NCORE = 8
SEG_PER_CORE = NSEG // NCORE        # 16
KT = L // 128                       # 2 position tiles per segment
DT = D // 128                       # 6 feature tiles
DW = D + 4                          # free pitch (cols 768:772 = 1.0)
SCALE = 1.0 / float(np.sqrt(D))
MAXB = 2                            # segments per compute batch / input group
NGRP = SEG_PER_CORE // MAXB         # 8 groups of 2 segments
OW = D + 1                          # output pitch: 768 numerator + denominator


def build_nc():
    nc = bass.Bass()
    x = nc.dram_tensor("x", [128, SEG_PER_CORE, KT, DW], BF16, kind="ExternalInput")
    xt = nc.dram_tensor(
        "xt", [128, SEG_PER_CORE, DT // 2, 2, L], FP8, kind="ExternalInput"
    )
    y = nc.dram_tensor("y", [128, SEG_PER_CORE, KT, OW], BF16, kind="ExternalOutput")
    Exp = mybir.ActivationFunctionType.Exp

    with tile.TileContext(nc) as tc:
        with (
            tc.tile_pool(name="xn", bufs=NGRP) as xn_pool,
            tc.tile_pool(name="xf", bufs=NGRP) as xf_pool,
            tc.tile_pool(name="e", bufs=SEG_PER_CORE) as e_pool,
            tc.tile_pool(name="osb", bufs=SEG_PER_CORE) as osb_pool,
            tc.tile_pool(name="ps", bufs=2, space="PSUM") as ps_pool,
        ):
            # static per-group input tiles (never recycled)
            xn_t = []
            xf_t = []
            for g in range(NGRP):
                xn_t.append(xn_pool.tile([128, MAXB, KT, DW], BF16, tag="xn", name=f"xn{g}"))
                xf_t.append(xf_pool.tile([128, MAXB, DT // 2, 2, L], FP8, tag="xf", name=f"xf{g}"))

            def issue_group(g):
                """Issue group g's input DMAs: even groups on the sync HWDGE
                ring, odd on scalar (xf as one 2-seg transfer, xn per-seg)."""
                eng = nc.sync if g % 2 == 0 else nc.scalar
                s0 = MAXB * g
                eng.dma_start(out=xf_t[g], in_=xt[:, s0 : s0 + MAXB])
                for k in range(MAXB):
                    eng.dma_start(out=xn_t[g][:, k], in_=x[:, s0 + k])

            # pre-loop: segments 0-3 alternate rings PER SEGMENT so both the
            # xf and xn of the earliest segments land as soon as possible
            # (one ring carrying xf0,xn0,xf1,xn1 serially would delay V0 by
            # ~3us). 8 transfers = exactly the HWDGE outstanding-sem window;
            # issuing more here would make later issue instructions block
            # their engine on the sem-window wait, which on scalar would
            # stall the exps queued behind them.
            for s in range(4):
                eng = nc.sync if s % 2 == 0 else nc.scalar
                eng.dma_start(out=xf_t[s // 2][:, s % 2], in_=xt[:, s])
                eng.dma_start(out=xn_t[s // 2][:, s % 2], in_=x[:, s])
            # remaining groups: paced 2 per batch, emitted AFTER the batch's
            # exps so scalar reaches them with the window-wait long satisfied
            issue_at = {0: (2, 3), 1: (4, 5), 2: (6, 7)}

            for g in range(NGRP):
                s0 = MAXB * g
                # ---- Q/K phase for both segments of the group
                es = []
                for sl in range(MAXB):
                    xfs = xf_t[g][:, sl]
                    sp = ps_pool.tile([128, 512], F32, tag="sp")
                    DR = mybir.MatmulPerfMode.DoubleRow
                    for kt in range(KT):
                        for j in range(DT // 2):
                            nc.tensor.matmul(
                                sp[:, kt * 256 : kt * 256 + 256],
                                xfs[:, j, :, kt * 128 : kt * 128 + 128],
                                xfs[:, j],
                                start=(j == 0),
                                stop=(j == DT // 2 - 1),
                                perf_mode=DR,
                                skip_group_check=(kt == 1),
                            )
                    e = e_pool.tile([128, 512], BF16, tag="e")
                    nc.scalar.activation(e[:], sp[:], Exp, scale=SCALE)
                    es.append(e)

                # paced input DMA issues (after exps, before V-phase copies)
                for gi in issue_at.get(g, ()):
                    issue_group(gi)

                # ---- V phase + eviction + per-segment store
                for sl in range(MAXB):
                    s = s0 + sl
                    e = es[sl]
                    xns = xn_t[g][:, sl]
                    osb = osb_pool.tile([128, KT, OW], BF16, tag="osb")
                    for qt in range(KT):
                        op0 = ps_pool.tile([128, 388], F32, tag="op0", bufs=3)
                        op1 = ps_pool.tile([128, 388], F32, tag="op1", bufs=3)
                        for kt in range(KT):
                            lhsT = e[:, kt * 256 + qt * 128 : kt * 256 + qt * 128 + 128]
                            nc.tensor.matmul(
                                op0[:, 0:384],
                                lhsT,
                                xns[:, kt, 0:384],
                                start=(kt == 0),
                                stop=(kt == KT - 1),
                            )
                            nc.tensor.matmul(
                                op1[:, 0:388],
                                lhsT,
                                xns[:, kt, 384:772],
                                start=(kt == 0),
                                stop=(kt == KT - 1),
                            )
                        dst = osb[:, qt]
                        # scalar is saturated with exps + DMA issues during
                        # batches 0-2; give it eviction work only once the
                        # input issues are done
                        if qt and g >= 3:
                            nc.scalar.copy(dst[:, 0:384], op0[:, 0:384])
                        else:
                            nc.vector.tensor_copy(dst[:, 0:384], op0[:, 0:384])
                        nc.vector.tensor_copy(dst[:, 384:769], op1[:, 0:385])

                    # per-segment store: two SWDGE queues for the bulk (each
                    # under the 8-deep SWDGE sem window), HWDGE rings for the
                    # tail (their FIFOs naturally finish input backlog first)
                    yv = y[:, s].rearrange("p t d -> p (t d)")
                    ov = osb.rearrange("p t d -> p (t d)")
                    if s < 12:
                        nc.gpsimd.dma_start(out=yv, in_=ov)
                    elif s % 2 == 0:
                        nc.sync.dma_start(out=yv, in_=ov)
                    else:
                        nc.scalar.dma_start(out=yv, in_=ov)
    return nc


def split_excess_waits(nc, max_waits=1):
    """This walrus build only encodes one sync wait per instruction; move
    excess waits onto preceding same-engine NOPs."""
    n_split = 0
    for fn in nc.m.functions:
        for blk in fn.blocks:
            insts = blk.instructions
            i = 0
            while i < len(insts):
                inst = insts[i]
                si = getattr(inst, "sync_info", None)
                waits = list(si.on_wait) if si and si.on_wait else []
                if len(waits) > max_waits:
                    nop = mybir.InstNoOp(name=f"I-waitsplit-{n_split}", ins=[], outs=[])
                    nop.engine = inst.engine
                    nop.sync_info = mybir.SyncInfo(
                        on_wait=waits[:max_waits], on_update=[]
                    )
                    inst.sync_info = mybir.SyncInfo(
                        on_wait=waits[max_waits:], on_update=list(si.on_update)
                    )
                    insts.insert(i, nop)
                    n_split += 1
                else:
                    i += 1
    return n_split


_NC = None


def _get_nc():
    global _NC
    if _NC is None:
        _NC = build_nc()
        split_excess_waits(_NC)
    return _NC


def shard_inputs(x):
    """Full x [4, 16384, 768] f32 -> 8 per-core dicts:
    x  [128, 16, 2, 772] bf16 (position-major + ones cols)
    xt [128, 16, 6, 2, 128] fp8e4m3 (feature-major)
    """
    xd = np.asarray(x).reshape(B, S_FULL // SEG, SEG, D)[:, :, ::DIL, :]
    xd = xd.reshape(NSEG, KT, 128, D)                 # [seg, t, p, d]
    xp = xd.transpose(2, 0, 1, 3)                     # [p, seg, t, d]
    xb = np.empty((128, NSEG, KT, DW), dtype=ml_dtypes.bfloat16)
    xb[..., 0:D] = xp.astype(ml_dtypes.bfloat16)
    xb[..., D:DW] = np.asarray(1.0, dtype=ml_dtypes.bfloat16)
    xt = (
        xb[..., 0:D]
        .reshape(128, NSEG, KT, DT // 2, 2, 128)      # [p, seg, t, j, c, dp]
        .transpose(5, 1, 3, 4, 2, 0)                  # [dp, seg, j, c, t, p]
        .reshape(128, NSEG, DT // 2, 2, L)            # [dp, seg, j, c, pos]
        .astype(ml_dtypes.float8_e4m3)
    )
    out = []
    for c in range(NCORE):
        sl = slice(SEG_PER_CORE * c, SEG_PER_CORE * (c + 1))
        out.append(
            {
                "x": np.ascontiguousarray(xb[:, sl]),
                "xt": np.ascontiguousarray(xt[:, sl]),
            }
        )
    return out


def assemble_output(results):
    ys = np.concatenate([results[c]["y"] for c in range(NCORE)], axis=1)
    ys = ys.astype(np.float32)                        # [p, seg, t, 769]
    num = ys[..., 0:D].transpose(1, 2, 0, 3)          # [seg, t, p, d]
    den = ys[..., D].transpose(1, 2, 0)[..., None]    # [seg, t, p, 1]
    out = num / den
    return np.ascontiguousarray(out.reshape(B, (S_FULL // SEG) * L, D))


def kernel(x):
    nc = _get_nc()
    in_maps = shard_inputs(x)
    core_ids = list(range(NCORE))
    # run twice: the first execution after a fresh NEFF load has been seen
    # returning unwritten output buffers; the repeat is cheap and reliable.
    run_bass_kernel_spmd(nc, in_maps, core_ids)
    res = run_bass_kernel_spmd(nc, in_maps, core_ids)
    return assemble_output(res.results)
